# revision 1
# baseline (speedup 1.0000x reference)
"""Trainium2 Bass kernel for nn_Net_LSV: neural local-stochastic-vol Monte Carlo.

Data-parallel over MC paths across 8 NeuronCores (2048 paths/core).
Layout per core: path p = g*128 + i  ->  partition i, chunk g (i in [0,128), g in [0,16)).
Per-path state lives as [128, 16]-shaped tiles; MLPs run feature-major
([feat, 2048], paths on the free dim) via PE mini-transposes of the state,
and MLP outputs fold back to path-land via per-chunk matmuls whose
stationary operand is the activation chunk.
"""
import numpy as np
from contextlib import ExitStack

import concourse.bass as bass
import concourse.bacc as bacc
import concourse.tile as tile
from concourse import mybir
from concourse.masks import make_identity
from concourse.bass_utils import run_bass_kernel_spmd

F32 = mybir.dt.float32
F32R = mybir.dt.float32r
AF = mybir.ActivationFunctionType
OP = mybir.AluOpType

N_CORES = 8
MC = 16384
P = 128
G = 16
MCC = P * G            # paths per core
NS = 21                # strikes
NM = 4                 # maturities
H1 = 100               # s_vol hidden
VH = 20                # vanilla hedge hidden
DV = 20                # v_drift / v_vol hidden


def build_program(steps, dbg_step=None):
    """steps: list of dicts with per-step baked constants:
       t0, h, sqh, rho_s, c_s, rate, idx (first live maturity), event (None or
       (ev_index, [k_slots])).  Returns (nc, names).
    """
    T = len(steps)
    n_ev = sum(1 for s in steps if s["event"] is not None)
    nc = bacc.Bacc()

    # ---------------- DRAM I/O ----------------
    z_d = nc.declare_dram_parameter("z_land", [P, T, G], F32, isOutput=False)
    zz_d = nc.declare_dram_parameter("zz_land", [P, T, G], F32, isOutput=False)
    w1sv_d = nc.declare_dram_parameter("w1sv", [2, H1], F32, isOutput=False)
    b1sv_d = nc.declare_dram_parameter("b1sv_tab", [H1, T], F32, isOutput=False)
    w2aug_d = nc.declare_dram_parameter("w2aug", [H1 + 1, H1], F32, isOutput=False)
    w3aug_d = nc.declare_dram_parameter("w3aug", [H1 + 1, 2], F32, isOutput=False)
    w1vh_d = nc.declare_dram_parameter("w1vh", [2, NM * VH], F32, isOutput=False)
    b1vh_d = nc.declare_dram_parameter("b1vh_tab", [NM * VH, T], F32, isOutput=False)
    w2vh_aug_d = nc.declare_dram_parameter("w2vh_aug", [NM * VH + 1, NM * NS], F32, isOutput=False)
    w1cat_d = nc.declare_dram_parameter("w1cat", [2, 2 * DV], F32, isOutput=False)
    b1cat_d = nc.declare_dram_parameter("b1cat", [2 * DV, 1], F32, isOutput=False)
    w2cat_aug_d = nc.declare_dram_parameter("w2cat_aug", [2 * DV + 1, 2], F32, isOutput=False)
    expb_d = nc.declare_dram_parameter("expb", [1, T], F32, isOutput=False)
    krep_d = nc.declare_dram_parameter("krep", [1, max(n_ev, 1) * NS], F32, isOutput=False)
    init_d = nc.declare_dram_parameter("initvals", [1, 4], F32, isOutput=False)
    out_d = nc.declare_dram_parameter("out", [2 * NM * NS], F32, isOutput=True)

    dbg = {}

    with tile.TileContext(nc) as tc, ExitStack() as ctx:
        stat = ctx.enter_context(tc.tile_pool(name="stat", bufs=1))
        work = ctx.enter_context(tc.tile_pool(name="work", bufs=2))
        ps_x2 = ctx.enter_context(tc.tile_pool(name="ps_x2", bufs=2, space="PSUM"))
        ps_mlp = ctx.enter_context(tc.tile_pool(name="ps_mlp", bufs=3, space="PSUM"))
        ps_cvf = ctx.enter_context(tc.tile_pool(name="ps_cvf", bufs=2, space="PSUM"))
        ps_pdv = ctx.enter_context(tc.tile_pool(name="ps_pdv", bufs=1, space="PSUM"))

        # ---------- static tiles ----------
        ident = stat.tile([P, P], F32)
        make_identity(nc, ident[:])
        zt = stat.tile([P, T, G], F32)
        nc.sync.dma_start(out=zt[:], in_=z_d[:])
        zzt = stat.tile([P, T, G], F32)
        nc.sync.dma_start(out=zzt[:], in_=zz_d[:])

        w1sv_f = stat.tile([2, H1], F32)
        nc.sync.dma_start(out=w1sv_f[:], in_=w1sv_d[:])
        w1sv = stat.tile([2, H1], F32R)
        nc.vector.tensor_copy(w1sv[:], w1sv_f[:])
        b1sv_tab = stat.tile([H1, T], F32)
        nc.sync.dma_start(out=b1sv_tab[:], in_=b1sv_d[:])
        w2aug_f = stat.tile([H1 + 1, H1], F32)
        nc.sync.dma_start(out=w2aug_f[:], in_=w2aug_d[:])
        w2aug = stat.tile([H1 + 1, H1], F32R)
        nc.vector.tensor_copy(w2aug[:], w2aug_f[:])
        w3aug_f = stat.tile([H1 + 1, 2], F32)
        nc.sync.dma_start(out=w3aug_f[:], in_=w3aug_d[:])
        w3aug = stat.tile([H1 + 1, 2], F32R)
        nc.vector.tensor_copy(w3aug[:], w3aug_f[:])
        w1vh_f = stat.tile([2, NM * VH], F32)
        nc.sync.dma_start(out=w1vh_f[:], in_=w1vh_d[:])
        w1vh = stat.tile([2, NM * VH], F32R)
        nc.vector.tensor_copy(w1vh[:], w1vh_f[:])
        b1vh_tab = stat.tile([NM * VH, T], F32)
        nc.sync.dma_start(out=b1vh_tab[:], in_=b1vh_d[:])
        w2vh_aug_f = stat.tile([NM * VH + 1, NM * NS], F32)
        nc.sync.dma_start(out=w2vh_aug_f[:], in_=w2vh_aug_d[:])
        w2vh_aug = stat.tile([NM * VH + 1, NM * NS], F32R)
        nc.vector.tensor_copy(w2vh_aug[:], w2vh_aug_f[:])
        w1cat_f = stat.tile([2, 2 * DV], F32)
        nc.sync.dma_start(out=w1cat_f[:], in_=w1cat_d[:])
        w1cat = stat.tile([2, 2 * DV], F32R)
        nc.vector.tensor_copy(w1cat[:], w1cat_f[:])
        b1cat = stat.tile([2 * DV, 1], F32)
        nc.sync.dma_start(out=b1cat[:], in_=b1cat_d[:])
        w2cat_aug_f = stat.tile([2 * DV + 1, 2], F32)
        nc.sync.dma_start(out=w2cat_aug_f[:], in_=w2cat_aug_d[:])
        w2cat_aug = stat.tile([2 * DV + 1, 2], F32R)
        nc.vector.tensor_copy(w2cat_aug[:], w2cat_aug_f[:])
        expb = stat.tile([P, T], F32)
        nc.sync.dma_start(out=expb[:], in_=expb_d[:].broadcast_to([P, T]))
        krep = stat.tile([P, max(n_ev, 1) * NS], F32)
        nc.sync.dma_start(out=krep[:], in_=krep_d[:].broadcast_to([P, max(n_ev, 1) * NS]))
        initv = stat.tile([P, 4], F32)
        nc.sync.dma_start(out=initv[:], in_=init_d[:].broadcast_to([P, 4]))

        bias0 = stat.tile([P, 1], F32)
        nc.gpsimd.memset(bias0[:], 0.0)
        ones_col = stat.tile([P, 1], F32)
        nc.gpsimd.memset(ones_col[:], 1.0)

        # ---------- persistent state ----------
        SV = stat.tile([P, 2 * G], F32)          # interleaved slog/v cols
        # init: slog = initv[:,0], v = initv[:,1] (broadcast cols)
        nc.vector.tensor_copy(SV[:, 0:2 * G:2], initv[:, 0:1].broadcast_to([P, G]))
        nc.vector.tensor_copy(SV[:, 1:2 * G:2], initv[:, 1:2].broadcast_to([P, G]))
        sd_a = stat.tile([P, G], F32)
        nc.vector.tensor_copy(sd_a[:], initv[:, 2:3].broadcast_to([P, G]))
        sd_b = stat.tile([P, G], F32)
        nc.gpsimd.memset(sd_b[:], 0.0)

        x2 = stat.tile([2, MCC], F32R)
        h1aug = stat.tile([H1 + 1, MCC], F32R)
        nc.vector.tensor_copy(h1aug[:], bias0[0:H1 + 1, :].broadcast_to([H1 + 1, MCC]))
        nc.vector.tensor_copy(x2[:], bias0[0:2, :].broadcast_to([2, MCC]))
        h2aug = stat.tile([H1 + 1, MCC], F32R)
        nc.vector.tensor_copy(h2aug[:], bias0[0:H1 + 1, :].broadcast_to([H1 + 1, MCC]))
        vh1aug = stat.tile([NM * VH + 1, MCC], F32R)
        nc.vector.tensor_copy(vh1aug[:], bias0[0:NM * VH + 1, :].broadcast_to([NM * VH + 1, MCC]))
        vv1aug = stat.tile([2 * DV + 1, MCC], F32R)
        nc.vector.tensor_copy(vv1aug[:], bias0[0:2 * DV + 1, :].broadcast_to([2 * DV + 1, MCC]))
        # ones rows of augmented activations
        nc.vector.tensor_copy(h1aug[96:H1 + 1, :], ones_col[96:H1 + 1, :].broadcast_to([5, MCC]))
        nc.vector.tensor_copy(h2aug[96:H1 + 1, :], ones_col[96:H1 + 1, :].broadcast_to([5, MCC]))
        nc.vector.tensor_copy(vh1aug[64:NM * VH + 1, :], ones_col[64:NM * VH + 1, :].broadcast_to([17, MCC]))
        nc.vector.tensor_copy(vv1aug[32:2 * DV + 1, :], ones_col[32:2 * DV + 1, :].broadcast_to([9, MCC]))

        cv = stat.tile([P, NM, G, NS], F32)
        nc.gpsimd.memset(cv[:], 0.0)
        cvfwd = stat.tile([P, NM, G, NS], F32)
        cvpre = stat.tile([P, NM, G, NS], F32)
        pd = stat.tile([P, G], F32)
        vvol = stat.tile([P, G], F32)
        outacc = stat.tile([1, 2 * NM * NS], F32)
        nc.gpsimd.memset(outacc[:], 0.0)

        pdv_ps = ps_pdv.tile([P, 4 * G], F32)    # cols 0..31 (pd,pad) pairs, 32..63 (vd,vv) pairs


        def softplus_(out_ap, in_ap, nparts, tagp):
            a = work.tile(list(in_ap.shape), F32, tag=tagp + "_a")
            nc.scalar.activation(a[:], in_ap, AF.Abs, bias=bias0[0:nparts, :], scale=1.0)
            e = work.tile(list(in_ap.shape), F32, tag=tagp + "_e")
            nc.scalar.activation(e[:], a[:], AF.Exp, bias=bias0[0:nparts, :], scale=-1.0)
            l = work.tile(list(in_ap.shape), F32, tag=tagp + "_l")
            nc.scalar.activation(l[:], e[:], AF.Ln, bias=ones_col[0:nparts, :], scale=1.0)
            r = work.tile(list(in_ap.shape), F32, tag=tagp + "_r")
            nc.vector.tensor_scalar(r[:], in_ap, 0.0, None, OP.max)
            nc.vector.tensor_add(out_ap, r[:], l[:])

        sd_tiles = [sd_a, sd_b]

        for t, st in enumerate(steps):
            t0, h, sqh = st["t0"], st["h"], st["sqh"]
            rho_s, c_s, rate = st["rho_s"], st["c_s"], st["rate"]
            idx = st["idx"]
            nlive = NM - idx
            sd_old = sd_tiles[t % 2]
            sd_new = sd_tiles[(t + 1) % 2]

            # ---- transpose state to feature rows: x2[0]=slog, x2[1]=v ----
            for q in range(4):
                x2q = ps_x2.tile([2, 512], F32, tag="x2q")
                for gg in range(4):
                    g = q * 4 + gg
                    nc.tensor.transpose(x2q[0:2, gg * P:(gg + 1) * P],
                                        SV[:, 2 * g:2 * g + 2], ident[:])
                if q % 2 == 0:
                    nc.vector.tensor_copy(x2[:, q * 512:(q + 1) * 512], x2q[:])
                else:
                    nc.scalar.copy(x2[:, q * 512:(q + 1) * 512], x2q[:])

            # ---- sv L1 ----
            for q in range(4):
                m = ps_mlp.tile([H1, 512], F32, tag="mlp")
                nc.tensor.matmul(m[:], w1sv[:], x2[:, q * 512:(q + 1) * 512])
                nc.scalar.activation(h1aug[0:H1, q * 512:(q + 1) * 512], m[:],
                                     AF.Relu, bias=b1sv_tab[:, t:t + 1], scale=1.0)
            # ---- vh L1 ----
            for q in range(4):
                m = ps_mlp.tile([NM * VH, 512], F32, tag="mlp")
                nc.tensor.matmul(m[:], w1vh[:], x2[:, q * 512:(q + 1) * 512])
                nc.scalar.activation(vh1aug[0:NM * VH, q * 512:(q + 1) * 512], m[:],
                                     AF.Relu, bias=b1vh_tab[:, t:t + 1], scale=1.0)
            # ---- vdvv L1 ----
            for q in range(4):
                m = ps_mlp.tile([2 * DV, 512], F32, tag="mlp")
                nc.tensor.matmul(m[:], w1cat[:], x2[:, q * 512:(q + 1) * 512])
                nc.scalar.activation(vv1aug[0:2 * DV, q * 512:(q + 1) * 512], m[:],
                                     AF.Relu, bias=b1cat[:], scale=1.0)
            # ---- sv L2 ----
            for q in range(4):
                m = ps_mlp.tile([H1, 512], F32, tag="mlp")
                nc.tensor.matmul(m[:], w2aug[:], h1aug[:, q * 512:(q + 1) * 512])
                nc.scalar.activation(h2aug[0:H1, q * 512:(q + 1) * 512], m[:],
                                     AF.Relu, bias=bias0[0:H1, :], scale=1.0)
            # ---- sv L3 fold: pd ----
            for g in range(G):
                nc.tensor.matmul(pdv_ps[:, 2 * g:2 * g + 2],
                                 h2aug[:, g * P:(g + 1) * P], w3aug[:])
            softplus_(pd[:], pdv_ps[:, 0:2 * G:2], P, "sp_pd")
            # ---- vdvv L2 fold: (vd, vv) pairs at cols 16+2g,17+2g ----
            for g in range(G):
                nc.tensor.matmul(pdv_ps[:, 2 * G + 2 * g:2 * G + 2 * g + 2],
                                 vv1aug[:, g * P:(g + 1) * P], w2cat_aug[:])
            vd_ap = pdv_ps[:, 2 * G:4 * G:2]
            softplus_(vvol[:], pdv_ps[:, 2 * G + 1:4 * G:2], P, "sp_vv")
            # ---- vh L2 fold (live maturities only): cvfwd ----
            for g in range(G):
                cps = ps_cvf.tile([P, NS * NM], F32, tag="cvf")
                nc.tensor.matmul(cps[:],
                                 vh1aug[:, g * P:(g + 1) * P],
                                 w2vh_aug[:])
                if g % 2 == 0:
                    nc.scalar.copy(cvpre[:, idx:NM, g, :], cps[:, idx * NS:NM * NS])
                else:
                    nc.vector.tensor_copy(cvpre[:, idx:NM, g, :], cps[:, idx * NS:NM * NS])

            softplus_(cvfwd[:, idx:NM].rearrange("p k g s -> p (k g s)"),
                      cvpre[:, idx:NM].rearrange("p k g s -> p (k g s)"), P, "sp_cv")

            # ---- state update ----
            z_t = zt[:, t, :]
            zz_t = zzt[:, t, :]
            tmp1 = work.tile([P, G], F32, tag="tmp1")
            nc.vector.tensor_scalar(tmp1[:], zz_t, float(c_s), None, OP.mult)
            dB = work.tile([P, G], F32, tag="dB")
            nc.vector.scalar_tensor_tensor(dB[:], z_t, float(rho_s), tmp1[:],
                                           OP.mult, OP.add)
            # V update
            vtmp = work.tile([P, G], F32, tag="vtmp")
            nc.vector.scalar_tensor_tensor(vtmp[:], vd_ap, float(h), SV[:, 1:2 * G:2],
                                           OP.mult, OP.add)
            vvdB = work.tile([P, G], F32, tag="vvdB")
            nc.vector.tensor_mul(vvdB[:], vvol[:], dB[:])
            # drift/diff pieces
            pd2 = work.tile([P, G], F32, tag="pd2")
            nc.vector.tensor_mul(pd2[:], pd[:], pd[:])
            drift = work.tile([P, G], F32, tag="drift")
            nc.vector.tensor_scalar(drift[:], pd2[:], -0.5, float(rate), OP.mult, OP.add)
            dc = work.tile([P, G], F32, tag="dc")
            nc.scalar.activation(dc[:], drift[:], AF.Abs, bias=bias0[:], scale=float(sqh))
            nc.vector.tensor_scalar(dc[:], dc[:], 1.0, None, OP.add)
            rcp1 = work.tile([P, G], F32, tag="rcp1")
            nc.vector.reciprocal(rcp1[:], dc[:])
            fc = work.tile([P, G], F32, tag="fc")
            nc.scalar.activation(fc[:], pd[:], AF.Abs, bias=bias0[:], scale=float(sqh))
            nc.vector.tensor_scalar(fc[:], fc[:], 1.0, None, OP.add)
            rcp2 = work.tile([P, G], F32, tag="rcp2")
            nc.vector.reciprocal(rcp2[:], fc[:])
            term1 = work.tile([P, G], F32, tag="term1")
            nc.vector.scalar_tensor_tensor(term1[:], drift[:], float(h), rcp1[:],
                                           OP.mult, OP.mult)
            diffz = work.tile([P, G], F32, tag="diffz")
            nc.vector.scalar_tensor_tensor(diffz[:], z_t, float(sqh), pd[:],
                                           OP.mult, OP.mult)
            term2 = work.tile([P, G], F32, tag="term2")
            nc.vector.tensor_mul(term2[:], diffz[:], rcp2[:])
            # Slog += term1 + term2  (in place on SV)
            nc.vector.tensor_add(SV[:, 0:2 * G:2], SV[:, 0:2 * G:2], term1[:])
            nc.vector.tensor_add(SV[:, 0:2 * G:2], SV[:, 0:2 * G:2], term2[:])
            # V = vtmp + vvdB  (in place on SV)
            nc.vector.tensor_add(SV[:, 1:2 * G:2], vtmp[:], vvdB[:])
            # discounted price Sd = exp(slog - r*t1)
            nc.scalar.activation(sd_new[:], SV[:, 0:2 * G:2], AF.Exp,
                                 bias=expb[:, t:t + 1], scale=1.0)
            dS = work.tile([P, G], F32, tag="dS")
            nc.vector.tensor_sub(dS[:], sd_new[:], sd_old[:])

            # ---- cv += cvfwd * dS  (live maturities) ----
            dS_b = dS[:].unsqueeze(1).unsqueeze(-1).broadcast_to([P, nlive, G, NS])
            cvds = work.tile([P, NM, G, NS], F32, tag="cvds")
            nc.vector.tensor_tensor(cvds[:, idx:NM], cvfwd[:, idx:NM], dS_b, OP.mult)
            nc.vector.tensor_add(cv[:, idx:NM], cv[:, idx:NM], cvds[:, idx:NM])

            # ---- maturity event ----
            if st["event"] is not None:
                ev, kslots = st["event"]
                pay = work.tile([P, G, NS], F32, tag="pay")
                sd_bc = sd_new[:].unsqueeze(-1).broadcast_to([P, G, NS])
                kd_bc = krep[:, ev * NS:(ev + 1) * NS].unsqueeze(1).broadcast_to([P, G, NS])
                nc.vector.tensor_tensor(pay[:], sd_bc, kd_bc, OP.subtract)
                nc.vector.tensor_scalar(pay[:], pay[:], 0.0, None, OP.max)
                price = work.tile([P, G, NS], F32, tag="price")
                nc.vector.tensor_sub(price[:], pay[:], cv[:, idx, :, :])
                price2 = work.tile([P, G, NS], F32, tag="price2")
                nc.vector.tensor_mul(price2[:], price[:], price[:])
                red = work.tile([P, 2 * NS], F32, tag="red")
                nc.vector.tensor_reduce(red[:, 0:NS], price[:].transpose([0, 2, 1]),
                                        mybir.AxisListType.X, OP.add)
                nc.vector.tensor_reduce(red[:, NS:2 * NS], price2[:].transpose([0, 2, 1]),
                                        mybir.AxisListType.X, OP.add)
                pred = ps_x2.tile([1, 2 * NS], F32, tag="x2q")
                nc.tensor.matmul(pred[:], ones_col[:], red[:])
                for k in kslots:
                    nc.scalar.copy(outacc[0:1, k * NS:(k + 1) * NS], pred[0:1, 0:NS])
                    nc.scalar.copy(outacc[0:1, NM * NS + k * NS:NM * NS + (k + 1) * NS],
                                   pred[0:1, NS:2 * NS])

            if dbg_step is not None and t == dbg_step:
                for nm, tl in [("dbg_sv", SV), ("dbg_pd", pd), ("dbg_vvol", vvol),
                               ("dbg_sd", sd_new), ("dbg_dS", dS)]:
                    d = nc.dram_tensor(nm, list(tl.shape), F32, kind="ExternalOutput")
                    nc.sync.dma_start(out=d[:], in_=tl[:])
                    dbg[nm] = d
                d = nc.dram_tensor("dbg_cvfwd", [P, NM, G, NS], F32, kind="ExternalOutput")
                nc.sync.dma_start(out=d[:], in_=cvfwd[:])
                d = nc.dram_tensor("dbg_cv", [P, NM, G, NS], F32, kind="ExternalOutput")
                nc.sync.dma_start(out=d[:], in_=cv[:])
                d = nc.dram_tensor("dbg_x2", [2, MCC], F32, kind="ExternalOutput")
                nc.sync.dma_start(out=d[:], in_=x2[:])

        nc.sync.dma_start(out=out_d[:].unsqueeze(0), in_=outacc[:])

    nc.compile()
    return nc


def _prep(inputs):
    """Host-side preprocessing -> (steps, arrays-for-in_maps, meta)."""
    f = lambda k: np.asarray(inputs[k], dtype=np.float32)
    S0 = float(f("S0")); rate = float(f("rate"))
    z = f("z"); zz = f("zz")
    timegrid = f("timegrid"); strikes = f("strikes")
    v0 = float(f("v0")[0]); rho = float(f("rho")[0])
    mats = np.asarray(inputs["maturities"]).astype(np.int64)

    rho_t = float(np.tanh(np.float32(rho)))
    c_t = float(np.sqrt(np.float32(1.0) - np.float32(rho_t) ** 2))
    V0 = float(1.0 / (1.0 + np.exp(-np.float32(v0))) * 0.5)
    slog0 = float(np.log(np.float32(S0)))

    days = np.round(timegrid * 365.0).astype(np.int64)
    T_all = len(timegrid) - 1
    le = days[1:, None] <= mats[None, :]
    idx_net = np.argmax(le, axis=1)
    is_mat = np.any(days[1:, None] == mats[None, :], axis=1)

    if not is_mat.any():
        return None

    T = int(np.max(np.nonzero(is_mat)[0])) + 1
    steps = []
    krep_list = []
    ev = 0
    for t in range(T):
        t0 = float(timegrid[t]); t1 = float(timegrid[t + 1])
        h = float(np.float32(t1) - np.float32(t0))
        sqh = float(np.sqrt(np.float32(h)))
        event = None
        if is_mat[t]:
            k = int(idx_net[t])
            # reference: sel = (mat_idx == idx) & im  -> only row idx written
            event = (ev, [k])
            krep_list.append(np.exp(-rate * t1).astype(np.float32) * strikes)
            ev += 1
        steps.append(dict(
            t0=t0, h=h, sqh=sqh, rho_s=rho_t * sqh, c_s=c_t * sqh, rate=rate,
            idx=int(idx_net[t]), event=event,
        ))

    # weight repacks
    sv_W1 = f("sv_W1"); sv_b1 = f("sv_b1"); sv_W2 = f("sv_W2"); sv_b2 = f("sv_b2")
    sv_W3 = f("sv_W3"); sv_b3 = f("sv_b3")
    vh_W1 = f("vh_W1"); vh_b1 = f("vh_b1"); vh_W2 = f("vh_W2"); vh_b2 = f("vh_b2")
    vd_W1 = f("vd_W1"); vd_b1 = f("vd_b1"); vd_W2 = f("vd_W2"); vd_b2 = f("vd_b2")
    vv_W1 = f("vv_W1"); vv_b1 = f("vv_b1"); vv_W2 = f("vv_W2"); vv_b2 = f("vv_b2")

    arrs = {}
    arrs["w1sv"] = np.ascontiguousarray(sv_W1[1:3, :])

    arrs["w2aug"] = np.concatenate([sv_W2, sv_b2[None, :]], 0)
    arrs["w3aug"] = np.concatenate([np.concatenate([sv_W3, sv_b3[None, :]], 0), np.zeros((H1 + 1, 1), np.float32)], 1)
    arrs["w1vh"] = np.concatenate([vh_W1[:, 1, :].reshape(1, NM * VH), np.zeros((1, NM * VH), np.float32)], 0)

    w2vh_aug = np.zeros((NM * VH + 1, NM * NS), np.float32)
    for k in range(NM):
        w2vh_aug[k * VH:(k + 1) * VH, k * NS:(k + 1) * NS] = vh_W2[k]
        w2vh_aug[NM * VH, k * NS:(k + 1) * NS] = vh_b2[k]
    arrs["w2vh_aug"] = w2vh_aug
    arrs["w1cat"] = np.concatenate([np.zeros((1, 2 * DV), np.float32), np.concatenate([vd_W1[0], vv_W1[0]])[None, :]], 0)
    arrs["b1cat"] = np.concatenate([vd_b1, vv_b1])[:, None]
    w2cat_aug = np.zeros((2 * DV + 1, 2), np.float32)
    w2cat_aug[0:DV, 0] = vd_W2[:, 0]
    w2cat_aug[DV:2 * DV, 1] = vv_W2[:, 0]
    w2cat_aug[2 * DV, 0] = vd_b2[0]
    w2cat_aug[2 * DV, 1] = vv_b2[0]
    arrs["w2cat_aug"] = w2cat_aug
    t0s = timegrid[:T].astype(np.float32)
    arrs["b1sv_tab"] = sv_b1[:, None] + np.outer(sv_W1[0, :], t0s)
    arrs["b1vh_tab"] = (vh_b1.reshape(NM * VH, 1)
                        + np.outer(vh_W1[:, 0, :].reshape(NM * VH), t0s))
    t1s = timegrid[1:T + 1]
    arrs["expb"] = np.ascontiguousarray((-rate * t1s)[None, :].astype(np.float32))
    if krep_list:
        arrs["krep"] = np.concatenate(krep_list)[None, :].astype(np.float32)
    else:
        arrs["krep"] = np.zeros((1, NS), np.float32)
    sd0 = float(np.exp(np.float32(slog0) - np.float32(rate) * timegrid[0]))
    arrs["initvals"] = np.array([[slog0, V0, sd0, 0.0]], np.float32)
    for k in arrs:
        arrs[k] = np.ascontiguousarray(arrs[k], dtype=np.float32)

    # z shards: [MCC, T] slice -> [G, P, T] -> [P, T, G]
    zshards, zzshards = [], []
    for c in range(N_CORES):
        for src, lst in ((z, zshards), (zz, zzshards)):
            s = src[c * MCC:(c + 1) * MCC, :T]
            s = s.reshape(G, P, T).transpose(1, 2, 0)
            lst.append(np.ascontiguousarray(s, dtype=np.float32))

    written = sorted({k for s in steps if s["event"] for k in s["event"][1]})
    return steps, arrs, zshards, zzshards, written, T


_CACHE = {}


def kernel(**inputs) -> np.ndarray:
    prep = _prep(inputs)
    if prep is None:
        return np.zeros((2, NM, NS), np.float32)
    steps, arrs, zshards, zzshards, written, T = prep

    key = (T,) + tuple(
        (s["t0"], s["h"], s["rho_s"], s["c_s"], s["rate"], s["idx"],
         None if s["event"] is None else (s["event"][0], tuple(s["event"][1])))
        for s in steps)
    nc = _CACHE.get(key)
    if nc is None:
        nc = build_program(steps)
        _CACHE[key] = nc

    in_maps = []
    for c in range(N_CORES):
        m = dict(arrs)
        m["z_land"] = zshards[c]
        m["zz_land"] = zzshards[c]
        in_maps.append(m)

    res = run_bass_kernel_spmd(nc, in_maps, list(range(N_CORES)))
    sums = np.zeros(2 * NM * NS, np.float64)
    for c in range(N_CORES):
        sums += res.results[c]["out"].astype(np.float64)
    s1 = sums[:NM * NS].reshape(NM, NS)
    s2 = sums[NM * NS:].reshape(NM, NS)
    pv = np.zeros((NM, NS), np.float64)
    pvar = np.zeros((NM, NS), np.float64)
    for k in written:
        pv[k] = s1[k] / MC
        pvar[k] = (s2[k] - MC * pv[k] ** 2) / (MC - 1)
    return np.stack([pv, pvar]).astype(np.float32)



# revision 15
# speedup vs baseline: 1.1490x; 1.1490x over previous
"""Trainium2 Bass kernel for nn_Net_LSV: neural local-stochastic-vol Monte Carlo.

Data-parallel over MC paths across 8 NeuronCores (2048 paths/core).
Layout per core: path p = g*128 + i  ->  partition i, chunk g (i in [0,128), g in [0,16)).

v2 design vs baseline:
- Biases folded into matmul stationaries via const-1 row of x2 / aug output
  cols, so all post-matmul copies are plain relu/copy and can run on any
  engine (scalar/vector/gpsimd) instead of only the Activation engine.
- Single activation-table working set {Exp, Ln, Abs, Relu, Copy} (the
  natural_log_exp_and_others table) -> no ACT_TABLE_LOAD thrash.
- Softplus(x) = Ln(1 + Exp(x)) in 2 passes (preacts are |x|<1 here).
- vh L2 + vd/vv L2 merged into one per-chunk fold matmul (stationary is the
  combined [121,128] activation chunk, moving [121,86]).
- pd/vv softplus on small [128,16] tiles first so the state-update chain
  starts early; the big cv softplus is off the critical path.
"""
import numpy as np
from contextlib import ExitStack

import concourse.bass as bass
import concourse.bacc as bacc
import concourse.tile as tile
from concourse import mybir
from concourse.masks import make_identity
from concourse.bass_utils import run_bass_kernel_spmd

F32 = mybir.dt.float32
F32R = mybir.dt.float32r
AF = mybir.ActivationFunctionType
OP = mybir.AluOpType

N_CORES = 8
MC = 16384
P = 128
G = 16
MCC = P * G            # paths per core
NS = 21                # strikes
NM = 4                 # maturities
H1 = 100               # s_vol hidden
L1SV = H1 + 1          # sv L1 output cols (incl ones col)
VH = 80                # vanilla hedge hidden (4 nets x 20)
DV = 40                # vd(20) + vv(20) hidden
L1VH = VH + 1 + DV     # combined vh/vdvv L1 output cols = 121
FW = NM * NS + 2       # fold moving cols: 84 cvpre + vd + vv


def build_program(steps):
    T = len(steps)
    n_ev = sum(1 for s in steps if s["event"] is not None)
    nc = bacc.Bacc()

    # ---------------- DRAM I/O ----------------
    z_d = nc.declare_dram_parameter("z_land", [P, T, G], F32, isOutput=False)
    zz_d = nc.declare_dram_parameter("zz_land", [P, T, G], F32, isOutput=False)
    w1sv_d = nc.declare_dram_parameter("w1sv_tab", [3, T * L1SV], F32R, isOutput=False)
    w1vh_d = nc.declare_dram_parameter("w1vh_tab", [3, T * L1VH], F32R, isOutput=False)
    w2aug_d = nc.declare_dram_parameter("w2aug", [L1SV, L1SV], F32R, isOutput=False)
    wcomb_d = nc.declare_dram_parameter("wcomb", [L1VH, FW], F32R, isOutput=False)
    w3aug_d = nc.declare_dram_parameter("w3aug", [L1SV, 2], F32R, isOutput=False)
    expb_d = nc.declare_dram_parameter("expb", [1, T], F32, isOutput=False)
    krep_d = nc.declare_dram_parameter("krep", [1, max(n_ev, 1) * NS], F32, isOutput=False)
    init_d = nc.declare_dram_parameter("initvals", [1, 4], F32, isOutput=False)
    out_d = nc.declare_dram_parameter("out", [2 * NM * NS], F32, isOutput=True)

    with tile.TileContext(nc) as tc, ExitStack() as ctx:
        stat = ctx.enter_context(tc.tile_pool(name="stat", bufs=1))
        work = ctx.enter_context(tc.tile_pool(name="work", bufs=2))
        ps_x2 = ctx.enter_context(tc.tile_pool(name="ps_x2", bufs=1, space="PSUM"))
        ps_l1 = ctx.enter_context(tc.tile_pool(name="ps_l1", bufs=1, space="PSUM"))
        ps_l2 = ctx.enter_context(tc.tile_pool(name="ps_l2", bufs=1, space="PSUM"))
        ps_fold = ctx.enter_context(tc.tile_pool(name="ps_fold", bufs=2, space="PSUM"))
        ps_pd = ctx.enter_context(tc.tile_pool(name="ps_pd", bufs=1, space="PSUM"))

        # ---------- static tiles ----------
        ident = stat.tile([P, P], F32)
        make_identity(nc, ident[:])
        zt = stat.tile([P, T, G], F32)
        nc.sync.dma_start(out=zt[:], in_=z_d[:])
        zzt = stat.tile([P, T, G], F32)
        nc.sync.dma_start(out=zzt[:], in_=zz_d[:])

        def load_r(dram, shape, tag):
            # f32r is bitwise f32 -> DMA straight in, no staging/cast
            r = stat.tile(shape, F32R, tag=tag)
            nc.sync.dma_start(out=r[:], in_=dram[:])
            return r

        w1sv = load_r(w1sv_d, [3, T * L1SV], "w1sv")
        w1vh = load_r(w1vh_d, [3, T * L1VH], "w1vh")
        w2aug = load_r(w2aug_d, [L1SV, L1SV], "w2aug")
        wcomb = load_r(wcomb_d, [L1VH, FW], "wcomb")
        w3aug = load_r(w3aug_d, [L1SV, 2], "w3aug")

        expb = stat.tile([P, T], F32)
        nc.sync.dma_start(out=expb[:], in_=expb_d[:].broadcast_to([P, T]))
        krep = stat.tile([P, max(n_ev, 1) * NS], F32)
        nc.sync.dma_start(out=krep[:], in_=krep_d[:].broadcast_to([P, max(n_ev, 1) * NS]))
        initv = stat.tile([P, 4], F32)
        nc.sync.dma_start(out=initv[:], in_=init_d[:].broadcast_to([P, 4]))

        bias0 = stat.tile([P, 1], F32)
        nc.gpsimd.memset(bias0[:], 0.0)
        ones_col = stat.tile([P, 1], F32)
        nc.gpsimd.memset(ones_col[:], 1.0)

        # ---------- persistent state ----------
        SV = stat.tile([P, 2 * G], F32)          # interleaved slog/v cols
        nc.vector.tensor_copy(SV[:, 0:2 * G:2], initv[:, 0:1].broadcast_to([P, G]))
        nc.vector.tensor_copy(SV[:, 1:2 * G:2], initv[:, 1:2].broadcast_to([P, G]))
        sd_a = stat.tile([P, G], F32)
        nc.vector.tensor_copy(sd_a[:], initv[:, 2:3].broadcast_to([P, G]))
        sd_b = stat.tile([P, G], F32)
        nc.gpsimd.memset(sd_b[:], 0.0)

        x2 = stat.tile([3, MCC], F32R)
        # fill with ones once; row 2 stays const, rows 0-1 rewritten per step
        nc.vector.tensor_copy(x2[:], ones_col[0:3, :].broadcast_to([3, MCC]))
        h1aug = stat.tile([L1SV, MCC], F32R)
        vhvv1 = stat.tile([L1VH, MCC], F32R)
        h2aug = stat.tile([L1SV, MCC], F32R)

        # cv stored with DESCENDING maturity: cv[:, j] is maturity k = NM-1-j,
        # so the live maturities are always the contiguous prefix [0:nlive).
        cv = stat.tile([P, NM, G, NS], F32)
        nc.gpsimd.memset(cv[:], 0.0)
        # fold_sb: per-chunk blocks of [vd, vv, cvpre(k=NM-1..idx)], Bc wide
        fold_sb = stat.tile([P, G * FW], F32)
        pdexp = stat.tile([P, G], F32)
        pd = stat.tile([P, G], F32)
        vvexp = stat.tile([P, G], F32)
        vvol = stat.tile([P, G], F32)
        outacc = stat.tile([1, 2 * NM * NS], F32)
        nc.gpsimd.memset(outacc[:], 0.0)

        sd_tiles = [sd_a, sd_b]

        for t, st in enumerate(steps):
            h, sqh = st["h"], st["sqh"]
            rho_s, c_s, rate = st["rho_s"], st["c_s"], st["rate"]
            idx = st["idx"]
            nlive = NM - idx
            B = nlive * NS
            sd_old = sd_tiles[t % 2]
            sd_new = sd_tiles[(t + 1) % 2]

            # ---- transpose state to feature rows: x2[0]=slog, x2[1]=v ----
            for q in range(4):
                x2q = ps_x2.tile([2, 512], F32, tag="x2q")
                for gg in range(4):
                    g = q * 4 + gg
                    nc.tensor.transpose(x2q[0:2, gg * P:(gg + 1) * P],
                                        SV[:, 2 * g:2 * g + 2], ident[:])
                if q % 2 == 0:
                    nc.vector.tensor_copy(x2[0:2, q * 512:(q + 1) * 512], x2q[:])
                else:
                    nc.scalar.copy(x2[0:2, q * 512:(q + 1) * 512], x2q[:])

            # ---- L1 (both nets, bias rows folded into per-step stationary) ----
            sv_stat = w1sv[:, t * L1SV:(t + 1) * L1SV]
            vh_stat = w1vh[:, t * L1VH:(t + 1) * L1VH]
            for q in range(4):
                m1 = ps_l1.tile([L1SV, 512], F32, tag="l1sv")
                nc.tensor.matmul(m1[:], sv_stat, x2[:, q * 512:(q + 1) * 512])
                if q % 2 == 0:
                    nc.scalar.activation(h1aug[:, q * 512:(q + 1) * 512], m1[:],
                                         AF.Relu, bias=bias0[0:L1SV, :], scale=1.0)
                else:
                    nc.vector.tensor_scalar(h1aug[:, q * 512:(q + 1) * 512], m1[:],
                                            0.0, None, OP.max)
                m2 = ps_l1.tile([L1VH, 512], F32, tag="l1vh")
                nc.tensor.matmul(m2[:], vh_stat, x2[:, q * 512:(q + 1) * 512])
                if q % 2 == 1:
                    nc.scalar.activation(vhvv1[:, q * 512:(q + 1) * 512], m2[:],
                                         AF.Relu, bias=bias0[0:L1VH, :], scale=1.0)
                else:
                    nc.vector.tensor_scalar(vhvv1[:, q * 512:(q + 1) * 512], m2[:],
                                            0.0, None, OP.max)

            # ---- sv L2 (aug col keeps ones row alive) ----
            for q in range(4):
                m = ps_l2.tile([L1SV, 512], F32, tag="l2")
                nc.tensor.matmul(m[:], w2aug[:], h1aug[:, q * 512:(q + 1) * 512])
                if q % 2 == 0:
                    nc.scalar.activation(h2aug[:, q * 512:(q + 1) * 512], m[:],
                                         AF.Relu, bias=bias0[0:L1SV, :], scale=1.0)
                else:
                    nc.vector.tensor_scalar(h2aug[:, q * 512:(q + 1) * 512], m[:],
                                            0.0, None, OP.max)

            # ---- pd fold: per-chunk [101,128] stationary x w3aug ----
            pdv_ps = ps_pd.tile([P, 2 * G], F32, tag="pdps")
            for g in range(G):
                nc.tensor.matmul(pdv_ps[:, 2 * g:2 * g + 2],
                                 h2aug[:, g * P:(g + 1) * P], w3aug[:])
            # pd softplus early (state chain depends on it)
            nc.scalar.activation(pdexp[:], pdv_ps[:, 0:2 * G:2], AF.Exp,
                                 bias=bias0[:], scale=1.0)
            nc.scalar.activation(pd[:], pdexp[:], AF.Ln,
                                 bias=ones_col[:], scale=1.0)

            # ---- combined fold: [vd, vv, cvpre desc-k] per chunk ----
            Bc = 2 + B
            for g in range(G):
                cps = ps_fold.tile([P, FW], F32, tag="cvf")
                nc.tensor.matmul(cps[:], vhvv1[:, g * P:(g + 1) * P], wcomb[:])
                # one contiguous copy: vd, vv, live cvpre -> fold_sb block g
                if g % 2 == 0:
                    nc.vector.tensor_copy(fold_sb[:, g * Bc:(g + 1) * Bc],
                                          cps[:, 0:Bc])
                else:
                    nc.scalar.copy(fold_sb[:, g * Bc:(g + 1) * Bc],
                                   cps[:, 0:Bc])

            vd_view = fold_sb[:, 0:G * Bc:Bc]
            vv_view = fold_sb[:, 1:G * Bc:Bc]
            # vv softplus early (V update depends on it)
            nc.scalar.activation(vvexp[:], vv_view, AF.Exp,
                                 bias=bias0[:], scale=1.0)
            nc.scalar.activation(vvol[:], vvexp[:], AF.Ln,
                                 bias=ones_col[:], scale=1.0)

            # ---- state update ----
            z_t = zt[:, t, :]
            zz_t = zzt[:, t, :]
            tmp1 = work.tile([P, G], F32, tag="tmp1")
            nc.vector.tensor_scalar(tmp1[:], zz_t, float(c_s), None, OP.mult)
            dB = work.tile([P, G], F32, tag="dB")
            nc.vector.scalar_tensor_tensor(dB[:], z_t, float(rho_s), tmp1[:],
                                           OP.mult, OP.add)
            vtmp = work.tile([P, G], F32, tag="vtmp")
            nc.vector.scalar_tensor_tensor(vtmp[:], vd_view, float(h),
                                           SV[:, 1:2 * G:2], OP.mult, OP.add)
            vvdB = work.tile([P, G], F32, tag="vvdB")
            nc.vector.tensor_mul(vvdB[:], vvol[:], dB[:])
            pd2 = work.tile([P, G], F32, tag="pd2")
            nc.vector.tensor_mul(pd2[:], pd[:], pd[:])
            drift = work.tile([P, G], F32, tag="drift")
            nc.vector.tensor_scalar(drift[:], pd2[:], -0.5, float(rate), OP.mult, OP.add)
            dc = work.tile([P, G], F32, tag="dc")
            nc.scalar.activation(dc[:], drift[:], AF.Abs, bias=bias0[:], scale=float(sqh))
            nc.vector.tensor_scalar(dc[:], dc[:], 1.0, None, OP.add)
            rcp1 = work.tile([P, G], F32, tag="rcp1")
            nc.vector.reciprocal(rcp1[:], dc[:])
            fc = work.tile([P, G], F32, tag="fc")
            nc.scalar.activation(fc[:], pd[:], AF.Abs, bias=bias0[:], scale=float(sqh))
            nc.vector.tensor_scalar(fc[:], fc[:], 1.0, None, OP.add)
            rcp2 = work.tile([P, G], F32, tag="rcp2")
            nc.vector.reciprocal(rcp2[:], fc[:])
            term1 = work.tile([P, G], F32, tag="term1")
            nc.vector.scalar_tensor_tensor(term1[:], drift[:], float(h), rcp1[:],
                                           OP.mult, OP.mult)
            diffz = work.tile([P, G], F32, tag="diffz")
            nc.vector.scalar_tensor_tensor(diffz[:], z_t, float(sqh), pd[:],
                                           OP.mult, OP.mult)
            term2 = work.tile([P, G], F32, tag="term2")
            nc.vector.tensor_mul(term2[:], diffz[:], rcp2[:])
            nc.vector.tensor_add(SV[:, 0:2 * G:2], SV[:, 0:2 * G:2], term1[:])
            nc.vector.tensor_add(SV[:, 0:2 * G:2], SV[:, 0:2 * G:2], term2[:])
            nc.vector.tensor_add(SV[:, 1:2 * G:2], vtmp[:], vvdB[:])
            nc.scalar.activation(sd_new[:], SV[:, 0:2 * G:2], AF.Exp,
                                 bias=expb[:, t:t + 1], scale=1.0)
            dS = work.tile([P, G], F32, tag="dS")
            nc.vector.tensor_sub(dS[:], sd_new[:], sd_old[:])

            # ---- big cv softplus: Exp then Ln over all live cvpre ----
            spv = fold_sb[:, 0:G * Bc].rearrange("p (g c) -> p g c", g=G)[:, :, 2:Bc]
            nc.scalar.activation(spv, spv, AF.Exp, bias=bias0[:], scale=1.0)
            nc.scalar.activation(spv, spv, AF.Ln, bias=ones_col[:], scale=1.0)

            # ---- cv += cvfwd * dS  (live maturities, desc-k prefix) ----
            cvf_view = spv.rearrange("p g (j s) -> p j g s", j=nlive)
            dS_b = dS[:].unsqueeze(1).unsqueeze(-1).broadcast_to([P, nlive, G, NS])
            cvds = work.tile([P, NM, G, NS], F32, tag="cvds")
            nc.gpsimd.tensor_tensor(cvds[:, 0:nlive], cvf_view, dS_b, OP.mult)
            nc.gpsimd.tensor_tensor(cv[:, 0:nlive], cv[:, 0:nlive],
                                    cvds[:, 0:nlive], OP.add)

            # ---- maturity event ----
            if st["event"] is not None:
                ev, kslots = st["event"]
                pay = work.tile([P, G, NS], F32, tag="pay")
                sd_bc = sd_new[:].unsqueeze(-1).broadcast_to([P, G, NS])
                kd_bc = krep[:, ev * NS:(ev + 1) * NS].unsqueeze(1).broadcast_to([P, G, NS])
                nc.vector.tensor_tensor(pay[:], sd_bc, kd_bc, OP.subtract)
                nc.vector.tensor_scalar(pay[:], pay[:], 0.0, None, OP.max)
                price = work.tile([P, G, NS], F32, tag="price")
                nc.vector.tensor_sub(price[:], pay[:], cv[:, nlive - 1, :, :])
                price2 = work.tile([P, G, NS], F32, tag="price2")
                nc.vector.tensor_mul(price2[:], price[:], price[:])
                red = work.tile([P, 2 * NS], F32, tag="red")
                nc.vector.tensor_reduce(red[:, 0:NS], price[:].transpose([0, 2, 1]),
                                        mybir.AxisListType.X, OP.add)
                nc.vector.tensor_reduce(red[:, NS:2 * NS], price2[:].transpose([0, 2, 1]),
                                        mybir.AxisListType.X, OP.add)
                pred = ps_x2.tile([1, 2 * NS], F32, tag="pred")
                nc.tensor.matmul(pred[:], ones_col[:], red[:])
                for k in kslots:
                    nc.scalar.copy(outacc[0:1, k * NS:(k + 1) * NS], pred[0:1, 0:NS])
                    nc.scalar.copy(outacc[0:1, NM * NS + k * NS:NM * NS + (k + 1) * NS],
                                   pred[0:1, NS:2 * NS])

        nc.sync.dma_start(out=out_d[:].unsqueeze(0), in_=outacc[:])

    nc.compile()
    return nc


def _prep(inputs):
    """Host-side preprocessing -> (steps, arrays-for-in_maps, shards, meta)."""
    f = lambda k: np.asarray(inputs[k], dtype=np.float32)
    S0 = float(f("S0")); rate = float(f("rate"))
    z = f("z"); zz = f("zz")
    timegrid = f("timegrid"); strikes = f("strikes")
    v0 = float(f("v0")[0]); rho = float(f("rho")[0])
    mats = np.asarray(inputs["maturities"]).astype(np.int64)

    rho_t = float(np.tanh(np.float32(rho)))
    c_t = float(np.sqrt(np.float32(1.0) - np.float32(rho_t) ** 2))
    V0 = float(1.0 / (1.0 + np.exp(-np.float32(v0))) * 0.5)
    slog0 = float(np.log(np.float32(S0)))

    days = np.round(timegrid * 365.0).astype(np.int64)
    le = days[1:, None] <= mats[None, :]
    idx_net = np.argmax(le, axis=1)
    is_mat = np.any(days[1:, None] == mats[None, :], axis=1)

    if not is_mat.any():
        return None

    T = int(np.max(np.nonzero(is_mat)[0])) + 1
    steps = []
    krep_list = []
    ev = 0
    for t in range(T):
        t0 = float(timegrid[t]); t1 = float(timegrid[t + 1])
        h = float(np.float32(t1) - np.float32(t0))
        sqh = float(np.sqrt(np.float32(h)))
        event = None
        if is_mat[t]:
            k = int(idx_net[t])
            event = (ev, [k])
            krep_list.append(np.exp(-rate * t1).astype(np.float32) * strikes)
            ev += 1
        steps.append(dict(
            t0=t0, h=h, sqh=sqh, rho_s=rho_t * sqh, c_s=c_t * sqh, rate=rate,
            idx=int(idx_net[t]), event=event,
        ))

    # ---- weight repacks ----
    sv_W1 = f("sv_W1"); sv_b1 = f("sv_b1"); sv_W2 = f("sv_W2"); sv_b2 = f("sv_b2")
    sv_W3 = f("sv_W3"); sv_b3 = f("sv_b3")
    vh_W1 = f("vh_W1"); vh_b1 = f("vh_b1"); vh_W2 = f("vh_W2"); vh_b2 = f("vh_b2")
    vd_W1 = f("vd_W1"); vd_b1 = f("vd_b1"); vd_W2 = f("vd_W2"); vd_b2 = f("vd_b2")
    vv_W1 = f("vv_W1"); vv_b1 = f("vv_b1"); vv_W2 = f("vv_W2"); vv_b2 = f("vv_b2")

    t0s = timegrid[:T].astype(np.float32)
    arrs = {}

    # w1sv_tab [3, T*101]: per t rows = [W1_slog; W1_v; b1 + W1_t*t0], col 100 = e3
    w1sv_tab = np.zeros((3, T, L1SV), np.float32)
    w1sv_tab[0, :, 0:H1] = sv_W1[1, :][None, :]
    w1sv_tab[1, :, 0:H1] = sv_W1[2, :][None, :]
    w1sv_tab[2, :, 0:H1] = sv_b1[None, :] + np.outer(t0s, sv_W1[0, :])
    w1sv_tab[2, :, H1] = 1.0
    arrs["w1sv_tab"] = w1sv_tab.reshape(3, T * L1SV)

    # w1vh_tab [3, T*121]: cols 0-79 vh nets, col 80 ones, cols 81-120 vd/vv
    w1vh_tab = np.zeros((3, T, L1VH), np.float32)
    w1vh_tab[0, :, 0:VH] = vh_W1[:, 1, :].reshape(VH)[None, :]
    w1vh_tab[2, :, 0:VH] = (vh_b1.reshape(VH)[None, :]
                            + np.outer(t0s, vh_W1[:, 0, :].reshape(VH)))
    w1vh_tab[2, :, VH] = 1.0
    w1vh_tab[1, :, VH + 1:VH + 1 + DV] = np.concatenate([vd_W1[0], vv_W1[0]])[None, :]
    w1vh_tab[2, :, VH + 1:VH + 1 + DV] = np.concatenate([vd_b1, vv_b1])[None, :]
    arrs["w1vh_tab"] = w1vh_tab.reshape(3, T * L1VH)

    # w2aug [101, 101]: [[W2, 0], [b2, 1]]
    w2aug = np.zeros((L1SV, L1SV), np.float32)
    w2aug[0:H1, 0:H1] = sv_W2
    w2aug[H1, 0:H1] = sv_b2
    w2aug[H1, H1] = 1.0
    arrs["w2aug"] = w2aug

    # wcomb [121, 86]: col 0 = vd, col 1 = vv, cols 2+ = cvpre in DESCENDING
    # maturity order (k = NM-1 first) so live maturities are a contiguous
    # prefix for any idx. vh_b2/vd_b2/vv_b2 ride on the ones row (VH).
    wcomb = np.zeros((L1VH, FW), np.float32)
    wcomb[VH + 1:VH + 1 + 20, 0] = vd_W2[:, 0]
    wcomb[VH, 0] = vd_b2[0]
    wcomb[VH + 21:VH + 41, 1] = vv_W2[:, 0]
    wcomb[VH, 1] = vv_b2[0]
    for k in range(NM):
        j = NM - 1 - k
        wcomb[k * 20:(k + 1) * 20, 2 + j * NS:2 + (j + 1) * NS] = vh_W2[k]
        wcomb[VH, 2 + j * NS:2 + (j + 1) * NS] = vh_b2[k]
    arrs["wcomb"] = wcomb

    # w3aug [101, 2]
    w3aug = np.zeros((L1SV, 2), np.float32)
    w3aug[0:H1, 0] = sv_W3[:, 0]
    w3aug[H1, 0] = sv_b3[0]
    arrs["w3aug"] = w3aug

    t1s = timegrid[1:T + 1]
    arrs["expb"] = np.ascontiguousarray((-rate * t1s)[None, :].astype(np.float32))
    if krep_list:
        arrs["krep"] = np.concatenate(krep_list)[None, :].astype(np.float32)
    else:
        arrs["krep"] = np.zeros((1, NS), np.float32)
    sd0 = float(np.exp(np.float32(slog0) - np.float32(rate) * timegrid[0]))
    arrs["initvals"] = np.array([[slog0, V0, sd0, 0.0]], np.float32)
    for k in arrs:
        arrs[k] = np.ascontiguousarray(arrs[k], dtype=np.float32)

    # z shards: [MCC, T] slice -> [G, P, T] -> [P, T, G]
    zshards, zzshards = [], []
    for c in range(N_CORES):
        for src, lst in ((z, zshards), (zz, zzshards)):
            s = src[c * MCC:(c + 1) * MCC, :T]
            s = s.reshape(G, P, T).transpose(1, 2, 0)
            lst.append(np.ascontiguousarray(s, dtype=np.float32))

    written = sorted({k for s in steps if s["event"] for k in s["event"][1]})
    return steps, arrs, zshards, zzshards, written, T


_CACHE = {}


def kernel(**inputs) -> np.ndarray:
    prep = _prep(inputs)
    if prep is None:
        return np.zeros((2, NM, NS), np.float32)
    steps, arrs, zshards, zzshards, written, T = prep

    key = (T,) + tuple(
        (s["t0"], s["h"], s["rho_s"], s["c_s"], s["rate"], s["idx"],
         None if s["event"] is None else (s["event"][0], tuple(s["event"][1])))
        for s in steps)
    nc = _CACHE.get(key)
    if nc is None:
        nc = build_program(steps)
        _CACHE[key] = nc

    in_maps = []
    for c in range(N_CORES):
        m = dict(arrs)
        m["z_land"] = zshards[c]
        m["zz_land"] = zzshards[c]
        in_maps.append(m)

    res = run_bass_kernel_spmd(nc, in_maps, list(range(N_CORES)))
    sums = np.zeros(2 * NM * NS, np.float64)
    for c in range(N_CORES):
        sums += res.results[c]["out"].astype(np.float64)
    s1 = sums[:NM * NS].reshape(NM, NS)
    s2 = sums[NM * NS:].reshape(NM, NS)
    pv = np.zeros((NM, NS), np.float64)
    pvar = np.zeros((NM, NS), np.float64)
    for k in written:
        pv[k] = s1[k] / MC
        pvar[k] = (s2[k] - MC * pv[k] ** 2) / (MC - 1)
    return np.stack([pv, pvar]).astype(np.float32)
